# revision 91
# baseline (speedup 1.0000x reference)
"""Trainium2 Bass kernel for nn_PlasticityModelMoE (8-core SPMD), v3.

Strategy (delta vs v2):
  Host precomputes everything input-only: wmod = w*sigmoid(delay) (fp8,
  b-major, with gate_W appended), conn*mask row (with the phase-1 fp8
  descale folded in), blend polynomial coefs, memory column-sums (for
  the exp-minus-one trick below). Device setup phases vanish; the head
  is just the wm8/xt8 DMAs on separate rings (~8us).

  Phase 1 (units-TP, 256/core) runs in fp8e4m3 DoubleRow over dk pairs
  (0.5 cyc/col vs bf16's 1.0): x pre-scaled x16, wmod x1024, descale
  folded into the gate-softmax exp scale and the conn*mask row. Gate
  softmax drops the max-subtract (logits are O(1)). Blend Horner chain
  in bf16 as before; fp8 blendT AllGather per chunk.

  Phase 3 unchanged (fp8 DoubleRow logitsT), but the exp now lands in
  f32 scratch and a DVE op emits fT = (exp-1)*64 in fp8. Phase 4 then
  computes read' = fT @ [mem8 | 1] in fp8 DoubleRow over mk pairs
  (memory pre-scaled x32 into fp8; the exact memory column-sums carry
  the exp(0)=1 bulk, so fp8 noise only rides the +/-0.2-magnitude f
  part). Epilogue: y = (r + csum*C4) / (s + 8192*C4).

  Tail: chunk 3's ReduceScatter is split in two so only a half-size RS
  is exposed past the last matmul; epilogues for chunk ch are emitted
  after phase3(ch+1) so they overlap compute instead of queueing at
  the end. The RS payload itself is fp8 (x1/1024 so the 8-way sum fits
  e4m3) — safe because only the f-part rides it.

  Scheduling: queues are kept wait-monotonic (sync = stagings then
  bT/rsinj/e_f in pipeline order; scalar = startup loads + ACT compute
  only; gpsimd = warmup + bulk loads + collective triggers — never
  data DMAs during the RS window, whose sw-DGE contends with the CC
  stream). bT(ch+1) fetches are emitted before rsinj(ch) staging so
  they never queue behind a phase-4 wait. tile_wait_until floors stop
  the scheduler hoisting post-AG triggers above phase-1 work.
"""
import numpy as np
import ml_dtypes
from contextlib import ExitStack

import concourse.bass as bass
import concourse.mybir as mybir
import concourse.tile as tile
from concourse import bacc
from concourse.bass_utils import run_bass_kernel_spmd
from concourse.masks import make_identity

F32 = mybir.dt.float32
BF16 = mybir.dt.bfloat16
F8 = mybir.dt.float8e4
AF = mybir.ActivationFunctionType
ALU = mybir.AluOpType
AX = mybir.AxisListType
DR = mybir.MatmulPerfMode.DoubleRow

XS = 16.0             # x -> fp8 pre-scale (|x| <~ 5 -> <~ 80)
WS = 1024.0           # wmod/gate_W -> fp8 pre-scale (|.| <~ 0.12 -> <~ 123)
C1 = XS * WS          # phase-1 matmul output scale
BLEND_SCALE = 64.0    # blend -> fp8 pre-scale (values ~[0.05, 0.45])
RW_SCALE = 256.0      # read_W -> fp8 pre-scale (values ~[-0.11, 0.11])
FS = 64.0             # f = exp(logit)-1 -> fp8 pre-scale (|f| <~ 2 -> <~ 128)
MSC = 32.0            # memory -> fp8 pre-scale (|mem| <~ 5.5 -> <~ 176)
C4 = FS * MSC         # phase-4 matmul output scale

KC = 8
N, D, U, NB, M, MD = 2048, 1024, 2048, 4, 8192, 1024
US = U // KC          # 256 units per core
MS = M // KC          # 1024 memory rows per core
NS = N // KC          # 256 output rows per core
DK = D // 128         # 8 k-tiles over D
UK = U // 128         # 16 k-tiles over U
MK = MS // 128        # 8 k-tiles over memory shard
UBF = US * NB         # 1024 branch columns per core
NCH = 4               # batch chunks
CW = N // NCH         # 512 columns per chunk
RS8 = 1024.0          # extra scale-down for the fp8 ReduceScatter payload
                      # (8 summed partials must stay inside e4m3 range)
DENOM = M * C4 / RS8  # added to the gathered s before reciprocal
GP = 32               # tiny matmul pad (gate block / s column) — moving
                      # operands narrower than this wedge the PE in DR mode
RS_GROUPS = [[4], [4], [4], [4]]  # sj per ReduceScatter, per chunk
# phase-3/4 processing order: chunk 2 last, so the tail past the final
# matmul is ONE ReduceScatter (its trigger is compute-gated either way;
# two half-size RSs cost more in per-op fixed time than one full one)
P34_ORDER = [0, 1, 3, 2]

_CMAT = np.array([
    [5.0000238e-01, 2.4987496e-01, 1.0582031e-03, -2.4046743e-02, 4.1678566e-03],
    [0.0, 1.0, 0.0, 0.0, 0.0],
    [-7.2632770e-06, 9.9976927e-01, 9.2018498e-03, -3.9401752e-01, 1.4669961e-01],
    [0.0, 1.0, 0.0, 0.0, 0.0],
    [8.6798245e-06, 4.9957812e-01, 2.5321743e-01, -8.1970906e-03, -1.3558048e-02],
    [3.9388153e-05, 4.9807969e-01, 4.1364601e-01, -3.7666172e-02, -3.2796454e-02],
    [0.0, 1.0507009873554805, 0.0, 0.0, 0.0],
    [3.1482985e-05, 5.9846270e-01, 3.3178753e-01, -4.6201140e-02, -1.9015398e-02],
    [0.0, 0.0, 0.0, 0.0, 0.0],
], dtype=np.float32)

_cache = {}


def _build(with_bias):
    nc = bacc.Bacc(num_devices=KC)

    xt_d = nc.dram_tensor("xt", [D, N], F8, kind="ExternalInput")
    wm_d = nc.dram_tensor("wm", [D, UBF + GP], F8, kind="ExternalInput")
    bias_d = nc.dram_tensor("bias", [UBF + NB], F32, kind="ExternalInput")
    coefs_d = nc.dram_tensor("coefs", [128, 5], F32, kind="ExternalInput")
    rw_d = nc.dram_tensor("rw", [U, MS], F8, kind="ExternalInput")
    rb_d = nc.dram_tensor("rb", [MS], F32, kind="ExternalInput")
    mem_d = nc.dram_tensor("mem", [MS, MD + GP], F8, kind="ExternalInput")
    csum_d = nc.dram_tensor("csum", [128, MD], F32, kind="ExternalInput")
    y_d = nc.dram_tensor("y", [NS, MD], F32, kind="ExternalOutput")

    with tile.TileContext(nc) as tc, ExitStack() as ctx:
        consts = ctx.enter_context(tc.tile_pool(name="consts", bufs=1))
        big = ctx.enter_context(tc.tile_pool(name="big", bufs=1))
        st = ctx.enter_context(tc.tile_pool(name="st", bufs=2))
        blendp = ctx.enter_context(tc.tile_pool(name="blendp", bufs=2))
        p34 = ctx.enter_context(tc.tile_pool(name="p34", bufs=2))
        dram_cc = ctx.enter_context(tc.tile_pool(name="dram_cc", bufs=1,
                                                 space="DRAM"))
        # PSUM budget (8 banks, bank-granular per tag-buf):
        #   br [128,1024] f32 = 2 banks x2 = 4  (ph1 branch; ph4 read)
        #   l  [128,512]  f32 = 1 bank  x2 = 2  (ph3 logits)
        #   sm [128,<=128]    = 1 bank  x2 = 2  (transposes, gate, s col)
        psum = ctx.enter_context(tc.tile_pool(name="psum", bufs=2, space="PSUM"))

        # --- gpsimd queue: warmup collective at t=0 (absorbs the CC
        # bring-up + inter-core launch skew off the AG0 path) ---
        warm_pool = ctx.enter_context(tc.tile_pool(name="warm_pool", bufs=1,
                                                   space="DRAM"))
        warm_sb = consts.tile([1, 16], BF16)
        nc.vector.memset(warm_sb, 0.0)
        warm_in = warm_pool.tile([1, 16], BF16, name="warm_in", tag="wi")
        nc.gpsimd.dma_start(out=warm_in, in_=warm_sb)
        warm_out = warm_pool.tile([KC, 16], BF16, name="warm_out", tag="wo",
                                  addr_space="Shared")
        nc.gpsimd.collective_compute(
            "AllGather", ALU.bypass, replica_groups=[list(range(KC))],
            ins=[warm_in.opt()], outs=[warm_out.opt()],
        )
        idf = consts.tile([128, 128], F32)
        make_identity(nc, idf)
        idb = consts.tile([128, 128], BF16)
        nc.any.tensor_copy(idb, idf)
        coefs = consts.tile([128, 5], F32)
        nc.gpsimd.dma_start(out=coefs, in_=coefs_d[:, :])
        rb_sb = consts.tile([128, MK], F32)
        nc.gpsimd.dma_start(out=rb_sb,
                            in_=rb_d.ap().rearrange("(t p) -> p t", p=128))
        if with_bias:
            bias_b = consts.tile([1, UBF + NB], F32)
            nc.gpsimd.dma_start(out=bias_b, in_=bias_d.ap()[None])
            ones_lhs = consts.tile([1, 128], BF16)
            nc.vector.memset(ones_lhs, 1.0)

        # --- gpsimd queue (pipeline spine): bulk ph3/4 loads after warmup ---
        rw_sb = big.tile([128, UK, MS], F8)
        for uk in range(UK):
            nc.gpsimd.dma_start(out=rw_sb[:, uk, :],
                                in_=rw_d[uk * 128:(uk + 1) * 128, :])
        mem_sb = big.tile([128, MK, MD + GP], F8)
        for mk in range(MK):
            nc.gpsimd.dma_start(out=mem_sb[:, mk, :],
                                in_=mem_d[mk * 128:(mk + 1) * 128, :])
        csum_bc = big.tile([128, MD], F32)
        nc.gpsimd.dma_start(out=csum_bc, in_=csum_d[:, :])

        # --- sync queue: wm8 now, then [stage0..3, bT0, rsinj0, bT1, ...]
        # in pipeline order so every wait on the queue is non-decreasing ---
        wm_sb = big.tile([128, DK, UBF + GP], F8)
        for dk in range(DK):
            nc.sync.dma_start(out=wm_sb[:, dk, :],
                              in_=wm_d[dk * 128:(dk + 1) * 128, :])

        # --- scalar queue: xt chunk 0 (critical), later chunks prefetched ---
        xt_c = [None] * NCH
        xt_c[0] = st.tile([128, DK, CW], F8, tag="xt", name="xt_c0")
        for dk in range(DK):
            nc.scalar.dma_start(out=xt_c[0][:, dk, :],
                                in_=xt_d[dk * 128:(dk + 1) * 128, 0:CW])

        # ---------------- pipeline state ----------------
        ag_outs = [None] * NCH
        ag_offs = [0] * NCH
        blendT_cs = [None] * NCH
        fTs = [None] * NCH
        rs_outs = [[None] * len(RS_GROUPS[ch]) for ch in range(NCH)]

        def phase1_tile(ch, t):
            csl = slice(t * 128, (t + 1) * 128)
            br = psum.tile([128, UBF], F32, tag="br", name="br")
            g_ps = psum.tile([128, GP], F32, tag="sm", name="g_ps")
            NP = DK // 2
            for p in range(NP):
                lhs = xt_c[ch][:, 2 * p:2 * p + 2, csl]
                start = (p == 0)
                stop = (p == NP - 1) and not with_bias
                # tiny gate MM first: the next pair's LDWEIGHTS then hides
                # under a 512-col stream instead of the tiny MM's drain
                nc.tensor.matmul(g_ps, lhs, wm_sb[:, 2 * p:2 * p + 2,
                                                  UBF:UBF + GP],
                                 start=start, stop=stop, perf_mode=DR)
                nc.tensor.matmul(br[:, 0:512], lhs,
                                 wm_sb[:, 2 * p:2 * p + 2, 0:512],
                                 start=start, stop=stop, perf_mode=DR)
                nc.tensor.matmul(br[:, 512:1024], lhs,
                                 wm_sb[:, 2 * p:2 * p + 2, 512:1024],
                                 start=start, stop=stop, perf_mode=DR)
            if with_bias:
                nc.tensor.matmul(br[:, 0:512], ones_lhs, bias_b[:, 0:512],
                                 start=False, stop=True)
                nc.tensor.matmul(br[:, 512:1024], ones_lhs,
                                 bias_b[:, 512:1024], start=False, stop=True)
                nc.tensor.matmul(g_ps[:, 0:NB], ones_lhs,
                                 bias_b[:, UBF:UBF + NB],
                                 start=False, stop=True)
            # gate softmax (no max-subtract: true logits are O(1))
            g_exp = blendp.tile([128, NB], F32, tag="g2")
            g_sum = blendp.tile([128, 1], F32, tag="g3")
            # conn*mask is folded into wm8 host-side, and the 1/C1 fp8
            # descale into the Horner coefficients (a stays C1-scaled; the
            # softmax ratio itself is scale-invariant)
            nc.scalar.activation(g_exp, g_ps[:, 0:NB], AF.Exp, scale=1.0 / C1,
                                 accum_out=g_sum)
            g_rec = blendp.tile([128, 1], F32, tag="g4")
            nc.vector.reciprocal(g_rec, g_sum)
            # z*gsum = sum_b eg_b * branch_b, fused into a 4-op
            # scalar_tensor_tensor chain spread over DVE and GpSimd;
            # the 1/gsum rides the relu's scale input below
            zt0 = blendp.tile([128, US], BF16, tag="t0")
            nc.vector.tensor_scalar_mul(zt0, br[:, 0:US], g_exp[:, 0:1])
            zt1 = blendp.tile([128, US], BF16, tag="t1")
            nc.vector.scalar_tensor_tensor(zt1, br[:, US:2 * US],
                                           g_exp[:, 1:2], zt0,
                                           ALU.mult, ALU.add)
            zt2 = blendp.tile([128, US], BF16, tag="t0")
            nc.vector.scalar_tensor_tensor(zt2, br[:, 2 * US:3 * US],
                                           g_exp[:, 2:3], zt1,
                                           ALU.mult, ALU.add)
            z_sb = blendp.tile([128, US], BF16, tag="t1")
            nc.vector.scalar_tensor_tensor(z_sb, br[:, 3 * US:4 * US],
                                           g_exp[:, 3:4], zt2,
                                           ALU.mult, ALU.add)
            # a = relu(z * conn * mask / gsum); conn*mask is pre-folded into
            # the wm8 branch columns, so no per-tile zc multiply is needed
            a_sb = blendp.tile([128, US], BF16, tag="ta")
            nc.scalar.activation(a_sb, z_sb, AF.Relu, scale=g_rec)
            # blend via degree-4 Horner (per-partition scalar coefs),
            # alternating engines so consecutive tiles' chains interleave
            hp = blendp.tile([128, US], BF16, tag="h2")
            nc.any.tensor_scalar(hp, a_sb, coefs[:, 4:5], coefs[:, 3:4],
                                 ALU.mult, ALU.add)
            hq = blendp.tile([128, US], BF16, tag="h3", bufs=1)
            nc.any.tensor_mul(hq, hp, a_sb)
            hr = blendp.tile([128, US], BF16, tag="h2")
            nc.any.tensor_scalar_add(hr, hq, coefs[:, 2:3])
            hs = blendp.tile([128, US], BF16, tag="h3", bufs=1)
            nc.any.tensor_mul(hs, hr, a_sb)
            ht = blendp.tile([128, US], BF16, tag="h2")
            nc.any.tensor_scalar_add(ht, hs, coefs[:, 1:2])
            hu = blendp.tile([128, US], BF16, tag="h3", bufs=1)
            nc.any.tensor_mul(hu, ht, a_sb)
            blend_sc = blendp.tile([128, US], BF16, tag="bb")
            nc.any.tensor_scalar(blend_sc, hu, coefs[:, 0:1], BLEND_SCALE,
                                 ALU.add, ALU.mult)
            blends[t] = blend_sc

        def phase1_transpose(t):
            # PE transposes for tile t, emitted AFTER tile t+1's matmuls so
            # they never stall the PE on tile t's DVE blend chain
            csl = slice(t * 128, (t + 1) * 128)
            for uh in range(2):
                trb_ps = psum.tile([128, 128], BF16, tag="sm")
                nc.tensor.transpose(trb_ps,
                                    blends[t][:, uh * 128:(uh + 1) * 128], idb)
                nc.scalar.activation(blendT_c[:, uh, csl], trb_ps, AF.Copy)

        def emit_allgather(ch):
            agi = dram_cc.tile([US, CW], F8, name=f"ag_in{ch}", tag=f"agi{ch}")
            for uh in range(2):
                nc.sync.dma_start(out=agi[uh * 128:(uh + 1) * 128, :],
                                  in_=blendT_cs[ch][:, uh, 0:CW])
            ago = dram_cc.tile([U, CW], F8, name=f"ag_out{ch}",
                               tag=f"ago{ch}", addr_space="Shared")
            nc.gpsimd.collective_compute(
                "AllGather", ALU.bypass,
                replica_groups=[list(range(KC))],
                ins=[agi.opt()], outs=[ago.opt()],
            )
            ag_outs[ch] = ago
            ag_offs[ch] = 0

        def emit_allgather23():
            # chunks 2+3 in ONE gather: each AG stretches by the cross-core
            # max of its staging time, and the last AG pays the widest
            # spread — paying it once instead of twice starts the RS chain
            # ~15us earlier
            agi = dram_cc.tile([US, 2 * CW], F8, name="ag_in23", tag="agi23")
            for ch in (2, 3):
                c0 = (ch - 2) * CW
                for uh in range(2):
                    nc.sync.dma_start(
                        out=agi[uh * 128:(uh + 1) * 128, c0:c0 + CW],
                        in_=blendT_cs[ch][:, uh, 0:CW])
            ago = dram_cc.tile([U, 2 * CW], F8, name="ag_out23",
                               tag="ago23", addr_space="Shared")
            nc.gpsimd.collective_compute(
                "AllGather", ALU.bypass,
                replica_groups=[list(range(KC))],
                ins=[agi.opt()], outs=[ago.opt()],
            )
            ag_outs[2] = ag_outs[3] = ago
            ag_offs[2] = 0
            ag_offs[3] = CW

        def fetch_bT(ch):
            # emitted BEFORE the previous chunk's rs_inj staging so this
            # fetch never queues behind a wait on phase-4 completion
            bT = p34.tile([128, UK, CW], F8, tag="bT", name="bT")
            for uk in range(UK):
                usl = slice(uk * 128, (uk + 1) * 128)
                # post-AG fetch is latency-critical: split across two rings
                # (safe: phase-1 ACT work retires before AG0 completes, so
                # these waits can't starve anything queued behind them)
                eng = nc.scalar if uk % 2 == 0 else nc.sync
                c0 = ag_offs[ch]
                eng.dma_start(out=bT[:, uk, :],
                              in_=ag_outs[ch][usl, c0:c0 + CW])
            return bT

        def phase3(ch, bT):
            fT_t = p34.tile([128, MK, CW], F8, tag="fT", name="fT_t")
            NP = UK // 2
            for mk in range(MK):
                l_ps = psum.tile([128, 512], F32, tag="l", name="l_ps")
                for p in range(NP):
                    nc.tensor.matmul(
                        l_ps,
                        rw_sb[:, 2 * p:2 * p + 2, mk * 128:(mk + 1) * 128],
                        bT[:, 2 * p:2 * p + 2, :],
                        start=(p == 0), stop=(p == NP - 1),
                        perf_mode=DR)
                e32 = p34.tile([128, CW], F32, tag="e32", name="e32")
                nc.scalar.activation(e32, l_ps, AF.Exp,
                                     bias=rb_sb[:, mk:mk + 1],
                                     scale=1.0 / (BLEND_SCALE * RW_SCALE))
                # fT = (exp - 1) * FS in fp8; the "1" bulk is carried by the
                # exact memory column-sums added in the epilogue
                nc.any.tensor_scalar(fT_t[:, mk, :], e32, 1.0, FS,
                                     ALU.subtract, ALU.mult)
            fTs[ch] = fT_t

        def phase4(ch):
            fT_t = fTs[ch]
            sj0 = 0
            for hf, spp in enumerate(RS_GROUPS[ch]):
                rs_inj = dram_cc.tile([spp * 128, MD + 1], F8,
                                      name=f"rs_in{ch}_{hf}",
                                      tag=f"rsi{ch}{hf}")
                for sj2 in range(spp):
                    sj = sj0 + sj2
                    jsl = slice(sj * 128, (sj + 1) * 128)
                    r_ps = psum.tile([128, MD], F32, tag="br", name="r_ps")
                    rs_ps = psum.tile([128, GP], F32, tag="sm", name="rs_ps")
                    NP = MK // 2
                    for p in range(NP):
                        stat = fT_t[:, 2 * p:2 * p + 2, jsl]
                        nc.tensor.matmul(rs_ps, stat,
                                         mem_sb[:, 2 * p:2 * p + 2,
                                                MD:MD + GP],
                                         start=(p == 0), stop=(p == NP - 1),
                                         perf_mode=DR)
                        nc.tensor.matmul(r_ps[:, 0:512], stat,
                                         mem_sb[:, 2 * p:2 * p + 2, 0:512],
                                         start=(p == 0), stop=(p == NP - 1),
                                         perf_mode=DR)
                        nc.tensor.matmul(r_ps[:, 512:1024], stat,
                                         mem_sb[:, 2 * p:2 * p + 2,
                                                512:1024],
                                         start=(p == 0), stop=(p == NP - 1),
                                         perf_mode=DR)
                    r_sb = p34.tile([128, MD + 1], F8, tag="rsb",
                                    name="r_sb")
                    # DVE, not ACT: the ACT queue must stay free for the
                    # next chunk's exps (l_ps anti-dep stalls the PE).
                    # 1/RS8 keeps the 8-way reduced sum inside e4m3 range;
                    # only the +/-0.2-magnitude f-part rides fp8 (the exp's
                    # bulk is carried exactly by the memory column-sums)
                    nc.vector.tensor_scalar_mul(r_sb[:, 0:MD], r_ps,
                                                1.0 / RS8)
                    nc.vector.tensor_scalar_mul(r_sb[:, MD:MD + 1],
                                                rs_ps[:, 0:1], 1.0 / RS8)
                    nc.sync.dma_start(out=rs_inj[sj2 * 128:(sj2 + 1) * 128, :],
                                      in_=r_sb)
                rs_out = dram_cc.tile([spp * 128 // KC, MD + 1], F8,
                                      name=f"rs_out{ch}_{hf}",
                                      tag=f"rso{ch}{hf}")
                nc.gpsimd.collective_compute(
                    "ReduceScatter", ALU.add,
                    replica_groups=[list(range(KC))],
                    ins=[rs_inj.opt()], outs=[rs_out.opt()],
                )
                rs_outs[ch][hf] = rs_out
                sj0 += spp

        def epilogue(ch, hf):
            rows = RS_GROUPS[ch][hf] * 128 // KC
            # sync-queue DMAs + DVE math only: gpsimd sw-DGE transfers
            # contend with the collective stream, and ACT must stay free
            e_f = p34.tile([rows, MD + 1], F8, tag="ef", name="e_f")
            nc.sync.dma_start(out=e_f, in_=rs_outs[ch][hf][:, :])
            s_pl = p34.tile([rows, 1], F32, tag="s32", name="s_pl")
            nc.vector.tensor_scalar_add(s_pl, e_f[:, MD:MD + 1], DENOM)
            s_rec = p34.tile([rows, 1], F32, tag="sr", name="s_rec")
            nc.vector.reciprocal(s_rec, s_pl)
            num = p34.tile([rows, MD], F32, tag="nm", name="num")
            nc.vector.scalar_tensor_tensor(num, e_f[:, 0:MD], 1.0,
                                           csum_bc[0:rows, :],
                                           ALU.mult, ALU.add)
            y_t = p34.tile([rows, MD], F32, tag="yt", name="y_t")
            nc.vector.tensor_scalar_mul(y_t, num, s_rec)
            r0 = ch * 64 + sum(RS_GROUPS[ch][:hf]) * 128 // KC
            nc.sync.dma_start(out=y_d[r0:r0 + rows, :], in_=y_t)

        # ---------------- main pipeline ----------------
        for ch in range(NCH):
            if ch + 1 < NCH:
                xt_c[ch + 1] = st.tile([128, DK, CW], F8, tag="xt",
                                       name=f"xt_c{ch + 1}")
                for dk in range(DK):
                    nc.scalar.dma_start(
                        out=xt_c[ch + 1][:, dk, :],
                        in_=xt_d[dk * 128:(dk + 1) * 128,
                                 (ch + 1) * CW:(ch + 2) * CW])
            blendT_c = blendp.tile([128, 2, CW], F8, tag="bl",
                                   name=f"blendT{ch}")
            blendT_cs[ch] = blendT_c
            blends = [None] * 4
            for t in range(4):
                phase1_tile(ch, t)
                if t >= 1:
                    phase1_transpose(t - 1)
            phase1_transpose(3)
            if ch <= 1:
                emit_allgather(ch)
            elif ch == 3:
                emit_allgather23()

        # all AG triggers precede all RS triggers on the collective queue,
        # so no early collective is blocked behind a late one's staging.
        # tile_wait_until floors (sim-time, conservative underestimates)
        # stop the scheduler hoisting post-AG work above phase-1 work on
        # shared queues.
        bTs = [None] * NCH
        with tc.tile_wait_until(0.060):
            bTs[P34_ORDER[0]] = fetch_bT(P34_ORDER[0])
        for i, ch in enumerate(P34_ORDER):
            with tc.tile_wait_until(0.060 + 0.020 * i):
                phase3(ch, bTs[ch])
            if i + 1 < NCH:
                with tc.tile_wait_until(0.062 + 0.020 * i):
                    nxt = P34_ORDER[i + 1]
                    bTs[nxt] = fetch_bT(nxt)
            if i >= 1:
                # previous chunk's RS is long done: fold its epilogue into
                # this chunk's compute window
                prev = P34_ORDER[i - 1]
                with tc.tile_wait_until(0.085 + 0.020 * i):
                    for hf in range(len(RS_GROUPS[prev])):
                        epilogue(prev, hf)
            with tc.tile_wait_until(0.070 + 0.020 * i):
                phase4(ch)
        last = P34_ORDER[-1]
        with tc.tile_wait_until(0.085 + 0.020 * NCH):
            for hf in range(len(RS_GROUPS[last])):
                epilogue(last, hf)

    nc.compile()
    return nc


def _make_in_maps(inputs):
    f8 = ml_dtypes.float8_e4m3
    x = np.asarray(inputs["x"], np.float32)
    w = np.asarray(inputs["w"], np.float32)
    delay = np.asarray(inputs["delay"], np.float32)
    b = np.asarray(inputs["b"], np.float32)
    gate_W = np.asarray(inputs["gate_W"], np.float32)
    gate_b = np.asarray(inputs["gate_b"], np.float32)
    na = np.asarray(inputs["neuron_avg"], np.float32)
    cw1 = np.asarray(inputs["conn_W1"], np.float32)
    cb1 = np.asarray(inputs["conn_b1"], np.float32)
    cw2 = np.asarray(inputs["conn_W2"], np.float32)
    cb2 = np.asarray(inputs["conn_b2"], np.float32)
    mask = np.asarray(inputs["mask"], np.float32)
    actw = np.asarray(inputs["act_w"], np.float64)
    read_W = np.asarray(inputs["read_W"], np.float32)
    read_b = np.asarray(inputs["read_b"], np.float32)
    mem = np.asarray(inputs["memory"], np.float32)

    # host precompute of all input-only terms
    xt8 = np.ascontiguousarray(x.T * XS).astype(f8)
    wmod = w * (1.0 / (1.0 + np.exp(-delay)))                   # [D,U,NB]
    h = np.maximum(na[None, :] @ cw1 + cb1, 0.0)
    conn = 1.0 / (1.0 + np.exp(-(h @ cw2 + cb2)))               # [1,U]
    cmw = conn[0] * mask                                        # [U]
    wmod = wmod * cmw[None, :, None]   # fold conn*mask into the weights
    ew = np.exp(actw - actw.max())
    wts = (ew / ew.sum()).astype(np.float32)
    coefs_row = (wts @ _CMAT.astype(np.float64))                     # [5]
    # a arrives C1-scaled (the fp8 descale is folded into the poly):
    # P(a/C1) = sum_i (c_i / C1^i) * a^i
    coefs_row = (coefs_row / (C1 ** np.arange(5))).astype(np.float32)
    coefs_t = np.ascontiguousarray(np.tile(coefs_row, (128, 1)))
    csum_t = np.ascontiguousarray(
        np.tile(mem.sum(axis=0, dtype=np.float64).astype(np.float32)
                * (C4 / RS8), (128, 1)))

    in_maps = []
    for k in range(KC):
        us, ue = k * US, (k + 1) * US
        ms, me = k * MS, (k + 1) * MS
        bias_row = np.concatenate(
            [(b[us:ue] * cmw[us:ue, None]).T.reshape(-1),
             gate_b]).astype(np.float32) * C1
        mem8 = np.concatenate(
            [mem[ms:me] * MSC, np.full((MS, 1), MSC, np.float32),
             np.zeros((MS, GP - 1), np.float32)],
            axis=1).astype(f8)
        in_maps.append({
            "xt": xt8,
            "wm": np.ascontiguousarray(np.concatenate(
                [wmod[:, us:ue, :].transpose(0, 2, 1).reshape(D, UBF) * WS,
                 gate_W * WS, np.zeros((D, GP - NB), np.float32)],
                axis=1)).astype(f8),
            "bias": np.ascontiguousarray(bias_row),
            "coefs": coefs_t,
            "rw": np.ascontiguousarray(read_W[:, ms:me] * RW_SCALE).astype(f8),
            "rb": np.ascontiguousarray(read_b[ms:me]),
            "mem": np.ascontiguousarray(mem8),
            "csum": csum_t,
        })
    return in_maps


def kernel(**inputs) -> np.ndarray:
    with_bias = bool(np.any(np.asarray(inputs["b"]))
                     or np.any(np.asarray(inputs["gate_b"])))
    key = ("nc", with_bias)
    if key not in _cache:
        _cache[key] = _build(with_bias)
        _cache["nc"] = _cache[key]
    nc = _cache[key]
    in_maps = _make_in_maps(inputs)
    res = run_bass_kernel_spmd(nc, in_maps, core_ids=list(range(KC)))
    out = np.empty((N, MD), np.float32)
    for k in range(KC):
        yk = res.results[k]["y"]
        for ch in range(NCH):
            base = 0
            for spp in RS_GROUPS[ch]:
                rows = spp * 128 // KC
                g0 = ch * 512 + base * 128 + k * rows
                l0 = ch * 64 + base * 128 // KC
                out[g0:g0 + rows] = yk[l0:l0 + rows]
                base += spp
    return out


# revision 95
# speedup vs baseline: 1.0309x; 1.0309x over previous
"""Trainium2 Bass kernel for nn_PlasticityModelMoE (8-core SPMD), v3.

Strategy (delta vs v2):
  Host precomputes everything input-only: wmod = w*sigmoid(delay) (fp8,
  b-major, with gate_W appended), conn*mask row (with the phase-1 fp8
  descale folded in), blend polynomial coefs, memory column-sums (for
  the exp-minus-one trick below). Device setup phases vanish; the head
  is just the wm8/xt8 DMAs on separate rings (~8us).

  Phase 1 (units-TP, 256/core) runs in fp8e4m3 DoubleRow over dk pairs
  (0.5 cyc/col vs bf16's 1.0): x pre-scaled x16, wmod x1024, descale
  folded into the gate-softmax exp scale and the conn*mask row. Gate
  softmax drops the max-subtract (logits are O(1)). Blend Horner chain
  in bf16 as before; fp8 blendT AllGather per chunk.

  Phase 3 unchanged (fp8 DoubleRow logitsT), but the exp now lands in
  f32 scratch and a DVE op emits fT = (exp-1)*64 in fp8. Phase 4 then
  computes read' = fT @ [mem8 | 1] in fp8 DoubleRow over mk pairs
  (memory pre-scaled x32 into fp8; the exact memory column-sums carry
  the exp(0)=1 bulk, so fp8 noise only rides the +/-0.2-magnitude f
  part). Epilogue: y = (r + csum*C4) / (s + 8192*C4).

  Tail: chunk 3's ReduceScatter is split in two so only a half-size RS
  is exposed past the last matmul; epilogues for chunk ch are emitted
  after phase3(ch+1) so they overlap compute instead of queueing at
  the end. The RS payload itself is fp8 (x1/1024 so the 8-way sum fits
  e4m3) — safe because only the f-part rides it.

  Scheduling: queues are kept wait-monotonic (sync = stagings then
  bT/rsinj/e_f in pipeline order; scalar = startup loads + ACT compute
  only; gpsimd = warmup + bulk loads + collective triggers — never
  data DMAs during the RS window, whose sw-DGE contends with the CC
  stream). bT(ch+1) fetches are emitted before rsinj(ch) staging so
  they never queue behind a phase-4 wait. tile_wait_until floors stop
  the scheduler hoisting post-AG triggers above phase-1 work.
"""
import numpy as np
import ml_dtypes
from contextlib import ExitStack

import concourse.bass as bass
import concourse.mybir as mybir
import concourse.tile as tile
from concourse import bacc
from concourse.bass_utils import run_bass_kernel_spmd
from concourse.masks import make_identity

F32 = mybir.dt.float32
BF16 = mybir.dt.bfloat16
F8 = mybir.dt.float8e4
AF = mybir.ActivationFunctionType
ALU = mybir.AluOpType
AX = mybir.AxisListType
DR = mybir.MatmulPerfMode.DoubleRow

XS = 16.0             # x -> fp8 pre-scale (|x| <~ 5 -> <~ 80)
WS = 1024.0           # wmod/gate_W -> fp8 pre-scale (|.| <~ 0.12 -> <~ 123)
C1 = XS * WS          # phase-1 matmul output scale
BLEND_SCALE = 64.0    # blend -> fp8 pre-scale (values ~[0.05, 0.45])
RW_SCALE = 256.0      # read_W -> fp8 pre-scale (values ~[-0.11, 0.11])
FS = 64.0             # f = exp(logit)-1 -> fp8 pre-scale (|f| <~ 2 -> <~ 128)
MSC = 32.0            # memory -> fp8 pre-scale (|mem| <~ 5.5 -> <~ 176)
C4 = FS * MSC         # phase-4 matmul output scale

KC = 8
N, D, U, NB, M, MD = 2048, 1024, 2048, 4, 8192, 1024
US = U // KC          # 256 units per core
MS = M // KC          # 1024 memory rows per core
NS = N // KC          # 256 output rows per core
DK = D // 128         # 8 k-tiles over D
UK = U // 128         # 16 k-tiles over U
MK = MS // 128        # 8 k-tiles over memory shard
UBF = US * NB         # 1024 branch columns per core
NCH = 4               # batch chunks
CW = N // NCH         # 512 columns per chunk
RS8 = 1024.0          # extra scale-down for the fp8 ReduceScatter payload
                      # (8 summed partials must stay inside e4m3 range)
DENOM = M * C4 / RS8  # added to the gathered s before reciprocal
GP = 32               # tiny matmul pad (gate block / s column) — moving
                      # operands narrower than this wedge the PE in DR mode
RS_GROUPS = [[4], [4], [4], [4]]  # sj per ReduceScatter, per chunk
# phase-3/4 processing order: chunk 2 last, so the tail past the final
# matmul is ONE ReduceScatter (its trigger is compute-gated either way;
# two half-size RSs cost more in per-op fixed time than one full one)
P34_ORDER = [0, 1, 3, 2]

_CMAT = np.array([
    [5.0000238e-01, 2.4987496e-01, 1.0582031e-03, -2.4046743e-02, 4.1678566e-03],
    [0.0, 1.0, 0.0, 0.0, 0.0],
    [-7.2632770e-06, 9.9976927e-01, 9.2018498e-03, -3.9401752e-01, 1.4669961e-01],
    [0.0, 1.0, 0.0, 0.0, 0.0],
    [8.6798245e-06, 4.9957812e-01, 2.5321743e-01, -8.1970906e-03, -1.3558048e-02],
    [3.9388153e-05, 4.9807969e-01, 4.1364601e-01, -3.7666172e-02, -3.2796454e-02],
    [0.0, 1.0507009873554805, 0.0, 0.0, 0.0],
    [3.1482985e-05, 5.9846270e-01, 3.3178753e-01, -4.6201140e-02, -1.9015398e-02],
    [0.0, 0.0, 0.0, 0.0, 0.0],
], dtype=np.float32)

_cache = {}


def _build(with_bias):
    nc = bacc.Bacc(num_devices=KC)

    xt_d = nc.dram_tensor("xt", [D, N], F8, kind="ExternalInput")
    wm_d = nc.dram_tensor("wm", [D, UBF + GP], F8, kind="ExternalInput")
    bias_d = nc.dram_tensor("bias", [UBF + NB], F32, kind="ExternalInput")
    coefs_d = nc.dram_tensor("coefs", [128, 5], F32, kind="ExternalInput")
    rw_d = nc.dram_tensor("rw", [U, MS], F8, kind="ExternalInput")
    rb_d = nc.dram_tensor("rb", [MS], F32, kind="ExternalInput")
    mem_d = nc.dram_tensor("mem", [MS, MD + GP], F8, kind="ExternalInput")
    csum_d = nc.dram_tensor("csum", [128, MD], F32, kind="ExternalInput")
    y_d = nc.dram_tensor("y", [NS, MD], F32, kind="ExternalOutput")

    with tile.TileContext(nc) as tc, ExitStack() as ctx:
        consts = ctx.enter_context(tc.tile_pool(name="consts", bufs=1))
        big = ctx.enter_context(tc.tile_pool(name="big", bufs=1))
        st = ctx.enter_context(tc.tile_pool(name="st", bufs=2))
        blendp = ctx.enter_context(tc.tile_pool(name="blendp", bufs=2))
        p34 = ctx.enter_context(tc.tile_pool(name="p34", bufs=2))
        dram_cc = ctx.enter_context(tc.tile_pool(name="dram_cc", bufs=1,
                                                 space="DRAM"))
        # PSUM budget (8 banks, bank-granular per tag-buf):
        #   br [128,1024] f32 = 2 banks x2 = 4  (ph1 branch; ph4 read)
        #   l  [128,512]  f32 = 1 bank  x2 = 2  (ph3 logits)
        #   sm [128,<=128]    = 1 bank  x2 = 2  (transposes, gate, s col)
        psum = ctx.enter_context(tc.tile_pool(name="psum", bufs=2, space="PSUM"))

        # --- gpsimd queue: warmup collective at t=0 (absorbs the CC
        # bring-up + inter-core launch skew off the AG0 path) ---
        warm_pool = ctx.enter_context(tc.tile_pool(name="warm_pool", bufs=1,
                                                   space="DRAM"))
        warm_sb = consts.tile([1, 16], BF16)
        nc.vector.memset(warm_sb, 0.0)
        warm_in = warm_pool.tile([1, 16], BF16, name="warm_in", tag="wi")
        nc.gpsimd.dma_start(out=warm_in, in_=warm_sb)
        warm_out = warm_pool.tile([KC, 16], BF16, name="warm_out", tag="wo",
                                  addr_space="Shared")
        nc.gpsimd.collective_compute(
            "AllGather", ALU.bypass, replica_groups=[list(range(KC))],
            ins=[warm_in.opt()], outs=[warm_out.opt()],
        )
        idf = consts.tile([128, 128], F32)
        make_identity(nc, idf)
        idb = consts.tile([128, 128], BF16)
        nc.any.tensor_copy(idb, idf)
        coefs = consts.tile([128, 5], F32)
        nc.gpsimd.dma_start(out=coefs, in_=coefs_d[:, :])
        rb_sb = consts.tile([128, MK], F32)
        nc.gpsimd.dma_start(out=rb_sb,
                            in_=rb_d.ap().rearrange("(t p) -> p t", p=128))
        if with_bias:
            bias_b = consts.tile([1, UBF + NB], F32)
            nc.gpsimd.dma_start(out=bias_b, in_=bias_d.ap()[None])
            ones_lhs = consts.tile([1, 128], BF16)
            nc.vector.memset(ones_lhs, 1.0)

        # --- gpsimd queue (pipeline spine): bulk ph3/4 loads after warmup ---
        rw_sb = big.tile([128, UK, MS], F8)
        for uk in range(UK):
            nc.gpsimd.dma_start(out=rw_sb[:, uk, :],
                                in_=rw_d[uk * 128:(uk + 1) * 128, :])
        mem_sb = big.tile([128, MK, MD + GP], F8)
        for mk in range(MK):
            nc.gpsimd.dma_start(out=mem_sb[:, mk, :],
                                in_=mem_d[mk * 128:(mk + 1) * 128, :])
        csum_bc = big.tile([128, MD], F32)
        nc.gpsimd.dma_start(out=csum_bc, in_=csum_d[:, :])

        # --- sync queue: wm8 now, then [stage0..3, bT0, rsinj0, bT1, ...]
        # in pipeline order so every wait on the queue is non-decreasing ---
        wm_sb = big.tile([128, DK, UBF + GP], F8)
        for dk in range(DK):
            nc.sync.dma_start(out=wm_sb[:, dk, :],
                              in_=wm_d[dk * 128:(dk + 1) * 128, :])

        # --- scalar queue: xt chunk 0 (critical), later chunks prefetched ---
        xt_c = [None] * NCH
        xt_c[0] = st.tile([128, DK, CW], F8, tag="xt", name="xt_c0")
        for dk in range(DK):
            nc.scalar.dma_start(out=xt_c[0][:, dk, :],
                                in_=xt_d[dk * 128:(dk + 1) * 128, 0:CW])

        # ---------------- pipeline state ----------------
        ag_outs = [None] * NCH
        ag_offs = [0] * NCH
        blendT_cs = [None] * NCH
        fTs = [None] * NCH
        rs_outs = [[None] * len(RS_GROUPS[ch]) for ch in range(NCH)]

        def phase1_tile(ch, t):
            csl = slice(t * 128, (t + 1) * 128)
            br = psum.tile([128, UBF], F32, tag="br", name="br")
            g_ps = psum.tile([128, GP], F32, tag="sm", name="g_ps")
            NP = DK // 2
            for p in range(NP):
                lhs = xt_c[ch][:, 2 * p:2 * p + 2, csl]
                start = (p == 0)
                stop = (p == NP - 1) and not with_bias
                # tiny gate MM first: the next pair's LDWEIGHTS then hides
                # under a 512-col stream instead of the tiny MM's drain
                nc.tensor.matmul(g_ps, lhs, wm_sb[:, 2 * p:2 * p + 2,
                                                  UBF:UBF + GP],
                                 start=start, stop=stop, perf_mode=DR)
                nc.tensor.matmul(br[:, 0:512], lhs,
                                 wm_sb[:, 2 * p:2 * p + 2, 0:512],
                                 start=start, stop=stop, perf_mode=DR)
                nc.tensor.matmul(br[:, 512:1024], lhs,
                                 wm_sb[:, 2 * p:2 * p + 2, 512:1024],
                                 start=start, stop=stop, perf_mode=DR)
            if with_bias:
                nc.tensor.matmul(br[:, 0:512], ones_lhs, bias_b[:, 0:512],
                                 start=False, stop=True)
                nc.tensor.matmul(br[:, 512:1024], ones_lhs,
                                 bias_b[:, 512:1024], start=False, stop=True)
                nc.tensor.matmul(g_ps[:, 0:NB], ones_lhs,
                                 bias_b[:, UBF:UBF + NB],
                                 start=False, stop=True)
            # gate softmax (no max-subtract: true logits are O(1))
            g_exp = blendp.tile([128, NB], F32, tag="g2")
            g_sum = blendp.tile([128, 1], F32, tag="g3")
            # conn*mask is folded into wm8 host-side, and the 1/C1 fp8
            # descale into the Horner coefficients (a stays C1-scaled; the
            # softmax ratio itself is scale-invariant)
            nc.scalar.activation(g_exp, g_ps[:, 0:NB], AF.Exp, scale=1.0 / C1,
                                 accum_out=g_sum)
            g_rec = blendp.tile([128, 1], F32, tag="g4")
            nc.vector.reciprocal(g_rec, g_sum)
            # z*gsum = sum_b eg_b * branch_b, fused into a 4-op
            # scalar_tensor_tensor chain spread over DVE and GpSimd;
            # the 1/gsum rides the relu's scale input below
            zt0 = blendp.tile([128, US], BF16, tag="t0")
            nc.vector.tensor_scalar_mul(zt0, br[:, 0:US], g_exp[:, 0:1])
            zt1 = blendp.tile([128, US], BF16, tag="t1")
            nc.vector.scalar_tensor_tensor(zt1, br[:, US:2 * US],
                                           g_exp[:, 1:2], zt0,
                                           ALU.mult, ALU.add)
            zt2 = blendp.tile([128, US], BF16, tag="t0")
            nc.vector.scalar_tensor_tensor(zt2, br[:, 2 * US:3 * US],
                                           g_exp[:, 2:3], zt1,
                                           ALU.mult, ALU.add)
            z_sb = blendp.tile([128, US], BF16, tag="t1")
            nc.vector.scalar_tensor_tensor(z_sb, br[:, 3 * US:4 * US],
                                           g_exp[:, 3:4], zt2,
                                           ALU.mult, ALU.add)
            # a = relu(z * conn * mask / gsum); conn*mask is pre-folded into
            # the wm8 branch columns, so no per-tile zc multiply is needed
            a_sb = blendp.tile([128, US], BF16, tag="ta")
            nc.scalar.activation(a_sb, z_sb, AF.Relu, scale=g_rec)
            # blend via degree-4 Horner (per-partition scalar coefs),
            # alternating engines so consecutive tiles' chains interleave
            hp = blendp.tile([128, US], BF16, tag="h2")
            nc.any.tensor_scalar(hp, a_sb, coefs[:, 4:5], coefs[:, 3:4],
                                 ALU.mult, ALU.add)
            hq = blendp.tile([128, US], BF16, tag="h3", bufs=1)
            nc.any.tensor_mul(hq, hp, a_sb)
            hr = blendp.tile([128, US], BF16, tag="h2")
            nc.any.tensor_scalar_add(hr, hq, coefs[:, 2:3])
            hs = blendp.tile([128, US], BF16, tag="h3", bufs=1)
            nc.any.tensor_mul(hs, hr, a_sb)
            ht = blendp.tile([128, US], BF16, tag="h2")
            nc.any.tensor_scalar_add(ht, hs, coefs[:, 1:2])
            hu = blendp.tile([128, US], BF16, tag="h3", bufs=1)
            nc.any.tensor_mul(hu, ht, a_sb)
            blend_sc = blendp.tile([128, US], BF16, tag="bb")
            nc.any.tensor_scalar(blend_sc, hu, coefs[:, 0:1], BLEND_SCALE,
                                 ALU.add, ALU.mult)
            blends[t] = blend_sc

        def phase1_transpose(t):
            # PE transposes for tile t, emitted AFTER tile t+1's matmuls so
            # they never stall the PE on tile t's DVE blend chain
            csl = slice(t * 128, (t + 1) * 128)
            for uh in range(2):
                trb_ps = psum.tile([128, 128], BF16, tag="sm")
                nc.tensor.transpose(trb_ps,
                                    blends[t][:, uh * 128:(uh + 1) * 128], idb)
                nc.scalar.activation(blendT_c[:, uh, csl], trb_ps, AF.Copy)

        def emit_allgather(ch):
            agi = dram_cc.tile([US, CW], F8, name=f"ag_in{ch}", tag=f"agi{ch}")
            for uh in range(2):
                nc.sync.dma_start(out=agi[uh * 128:(uh + 1) * 128, :],
                                  in_=blendT_cs[ch][:, uh, 0:CW])
            ago = dram_cc.tile([U, CW], F8, name=f"ag_out{ch}",
                               tag=f"ago{ch}", addr_space="Shared")
            nc.gpsimd.collective_compute(
                "AllGather", ALU.bypass,
                replica_groups=[list(range(KC))],
                ins=[agi.opt()], outs=[ago.opt()],
            )
            ag_outs[ch] = ago
            ag_offs[ch] = 0

        def emit_allgather23():
            # chunks 2+3 in ONE gather: each AG stretches by the cross-core
            # max of its staging time, and the last AG pays the widest
            # spread — paying it once instead of twice starts the RS chain
            # ~15us earlier
            agi = dram_cc.tile([US, 2 * CW], F8, name="ag_in23", tag="agi23")
            for ch in (2, 3):
                c0 = (ch - 2) * CW
                for uh in range(2):
                    nc.sync.dma_start(
                        out=agi[uh * 128:(uh + 1) * 128, c0:c0 + CW],
                        in_=blendT_cs[ch][:, uh, 0:CW])
            ago = dram_cc.tile([U, 2 * CW], F8, name="ag_out23",
                               tag="ago23", addr_space="Shared")
            nc.gpsimd.collective_compute(
                "AllGather", ALU.bypass,
                replica_groups=[list(range(KC))],
                ins=[agi.opt()], outs=[ago.opt()],
            )
            ag_outs[2] = ag_outs[3] = ago
            ag_offs[2] = 0
            ag_offs[3] = CW

        def fetch_bT(ch):
            # emitted BEFORE the previous chunk's rs_inj staging so this
            # fetch never queues behind a wait on phase-4 completion
            bT = p34.tile([128, UK, CW], F8, tag="bT", name="bT")
            for uk in range(UK):
                usl = slice(uk * 128, (uk + 1) * 128)
                # post-AG fetch is latency-critical: split across two rings
                # (safe: phase-1 ACT work retires before AG0 completes, so
                # these waits can't starve anything queued behind them)
                eng = nc.scalar if uk % 2 == 0 else nc.sync
                c0 = ag_offs[ch]
                eng.dma_start(out=bT[:, uk, :],
                              in_=ag_outs[ch][usl, c0:c0 + CW])
            return bT

        def phase3(ch, bT):
            fT_t = p34.tile([128, MK, CW], F8, tag="fT", name="fT_t")
            NP = UK // 2
            for mk in range(MK):
                l_ps = psum.tile([128, 512], F32, tag="l", name="l_ps")
                for p in range(NP):
                    nc.tensor.matmul(
                        l_ps,
                        rw_sb[:, 2 * p:2 * p + 2, mk * 128:(mk + 1) * 128],
                        bT[:, 2 * p:2 * p + 2, :],
                        start=(p == 0), stop=(p == NP - 1),
                        perf_mode=DR)
                e32 = p34.tile([128, CW], F32, tag="e32", name="e32")
                nc.scalar.activation(e32, l_ps, AF.Exp,
                                     bias=rb_sb[:, mk:mk + 1],
                                     scale=1.0 / (BLEND_SCALE * RW_SCALE))
                # fT = (exp - 1) * FS in fp8; the "1" bulk is carried by the
                # exact memory column-sums added in the epilogue
                nc.any.tensor_scalar(fT_t[:, mk, :], e32, 1.0, FS,
                                     ALU.subtract, ALU.mult)
            fTs[ch] = fT_t

        def phase4(ch):
            fT_t = fTs[ch]
            sj0 = 0
            for hf, spp in enumerate(RS_GROUPS[ch]):
                rs_inj = dram_cc.tile([spp * 128, MD + 1], F8,
                                      name=f"rs_in{ch}_{hf}",
                                      tag=f"rsi{ch}{hf}")
                for sj2 in range(spp):
                    sj = sj0 + sj2
                    jsl = slice(sj * 128, (sj + 1) * 128)
                    r_ps = psum.tile([128, MD], F32, tag="br", name="r_ps")
                    rs_ps = psum.tile([128, GP], F32, tag="sm", name="rs_ps")
                    NP = MK // 2
                    for p in range(NP):
                        stat = fT_t[:, 2 * p:2 * p + 2, jsl]
                        nc.tensor.matmul(rs_ps, stat,
                                         mem_sb[:, 2 * p:2 * p + 2,
                                                MD:MD + GP],
                                         start=(p == 0), stop=(p == NP - 1),
                                         perf_mode=DR)
                        nc.tensor.matmul(r_ps[:, 0:512], stat,
                                         mem_sb[:, 2 * p:2 * p + 2, 0:512],
                                         start=(p == 0), stop=(p == NP - 1),
                                         perf_mode=DR)
                        nc.tensor.matmul(r_ps[:, 512:1024], stat,
                                         mem_sb[:, 2 * p:2 * p + 2,
                                                512:1024],
                                         start=(p == 0), stop=(p == NP - 1),
                                         perf_mode=DR)
                    r_sb = p34.tile([128, MD + 1], F8, tag="rsb",
                                    name="r_sb")
                    # DVE, not ACT: the ACT queue must stay free for the
                    # next chunk's exps (l_ps anti-dep stalls the PE).
                    # 1/RS8 keeps the 8-way reduced sum inside e4m3 range;
                    # only the +/-0.2-magnitude f-part rides fp8 (the exp's
                    # bulk is carried exactly by the memory column-sums)
                    nc.vector.tensor_scalar_mul(r_sb[:, 0:MD], r_ps,
                                                1.0 / RS8)
                    nc.vector.tensor_scalar_mul(r_sb[:, MD:MD + 1],
                                                rs_ps[:, 0:1], 1.0 / RS8)
                    nc.sync.dma_start(out=rs_inj[sj2 * 128:(sj2 + 1) * 128, :],
                                      in_=r_sb)
                rs_out = dram_cc.tile([spp * 128 // KC, MD + 1], F8,
                                      name=f"rs_out{ch}_{hf}",
                                      tag=f"rso{ch}{hf}")
                nc.gpsimd.collective_compute(
                    "ReduceScatter", ALU.add,
                    replica_groups=[list(range(KC))],
                    ins=[rs_inj.opt()], outs=[rs_out.opt()],
                )
                rs_outs[ch][hf] = rs_out
                sj0 += spp

        def epilogue(ch, hf):
            rows = RS_GROUPS[ch][hf] * 128 // KC
            # sync-queue DMAs + DVE math only: gpsimd sw-DGE transfers
            # contend with the collective stream, and ACT must stay free
            e_f = p34.tile([rows, MD + 1], F8, tag="ef", name="e_f")
            nc.sync.dma_start(out=e_f, in_=rs_outs[ch][hf][:, :])
            s_pl = p34.tile([rows, 1], F32, tag="s32", name="s_pl")
            nc.vector.tensor_scalar_add(s_pl, e_f[:, MD:MD + 1], DENOM)
            s_rec = p34.tile([rows, 1], F32, tag="sr", name="s_rec")
            nc.vector.reciprocal(s_rec, s_pl)
            num = p34.tile([rows, MD], F32, tag="nm", name="num")
            nc.vector.scalar_tensor_tensor(num, e_f[:, 0:MD], 1.0,
                                           csum_bc[0:rows, :],
                                           ALU.mult, ALU.add)
            y_t = p34.tile([rows, MD], F32, tag="yt", name="y_t")
            nc.vector.tensor_scalar_mul(y_t, num, s_rec)
            r0 = ch * 64 + sum(RS_GROUPS[ch][:hf]) * 128 // KC
            nc.sync.dma_start(out=y_d[r0:r0 + rows, :], in_=y_t)

        # ---------------- main pipeline ----------------
        for ch in range(NCH):
            if ch + 1 < NCH:
                xt_c[ch + 1] = st.tile([128, DK, CW], F8, tag="xt",
                                       name=f"xt_c{ch + 1}")
                for dk in range(DK):
                    nc.scalar.dma_start(
                        out=xt_c[ch + 1][:, dk, :],
                        in_=xt_d[dk * 128:(dk + 1) * 128,
                                 (ch + 1) * CW:(ch + 2) * CW])
            blendT_c = blendp.tile([128, 2, CW], F8, tag="bl",
                                   name=f"blendT{ch}")
            blendT_cs[ch] = blendT_c
            blends = [None] * 4
            for t in range(4):
                phase1_tile(ch, t)
                if t >= 1:
                    phase1_transpose(t - 1)
            phase1_transpose(3)
            if ch <= 1:
                emit_allgather(ch)
            elif ch == 3:
                emit_allgather23()

        # all AG triggers precede all RS triggers on the collective queue,
        # so no early collective is blocked behind a late one's staging.
        # tile_wait_until floors (sim-time, conservative underestimates)
        # stop the scheduler hoisting post-AG work above phase-1 work on
        # shared queues.
        bTs = [None] * NCH
        with tc.tile_wait_until(0.060):
            bTs[P34_ORDER[0]] = fetch_bT(P34_ORDER[0])
        for i, ch in enumerate(P34_ORDER):
            with tc.tile_wait_until(0.060 + 0.020 * i):
                phase3(ch, bTs[ch])
            if i + 1 < NCH:
                with tc.tile_wait_until(0.062 + 0.020 * i):
                    nxt = P34_ORDER[i + 1]
                    bTs[nxt] = fetch_bT(nxt)
            if i >= 1:
                # previous chunk's RS is long done: fold its epilogue into
                # this chunk's compute window
                prev = P34_ORDER[i - 1]
                with tc.tile_wait_until(0.085 + 0.020 * i):
                    for hf in range(len(RS_GROUPS[prev])):
                        epilogue(prev, hf)
            with tc.tile_wait_until(0.070 + 0.020 * i):
                phase4(ch)
        last = P34_ORDER[-1]
        with tc.tile_wait_until(0.085 + 0.020 * NCH):
            for hf in range(len(RS_GROUPS[last])):
                epilogue(last, hf)

    nc.compile()
    return nc


def _make_in_maps(inputs):
    f8 = ml_dtypes.float8_e4m3
    x = np.asarray(inputs["x"], np.float32)
    w = np.asarray(inputs["w"], np.float32)
    delay = np.asarray(inputs["delay"], np.float32)
    b = np.asarray(inputs["b"], np.float32)
    gate_W = np.asarray(inputs["gate_W"], np.float32)
    gate_b = np.asarray(inputs["gate_b"], np.float32)
    na = np.asarray(inputs["neuron_avg"], np.float32)
    cw1 = np.asarray(inputs["conn_W1"], np.float32)
    cb1 = np.asarray(inputs["conn_b1"], np.float32)
    cw2 = np.asarray(inputs["conn_W2"], np.float32)
    cb2 = np.asarray(inputs["conn_b2"], np.float32)
    mask = np.asarray(inputs["mask"], np.float32)
    actw = np.asarray(inputs["act_w"], np.float64)
    read_W = np.asarray(inputs["read_W"], np.float32)
    read_b = np.asarray(inputs["read_b"], np.float32)
    mem = np.asarray(inputs["memory"], np.float32)

    # host precompute of all input-only terms
    xt8 = np.ascontiguousarray(x.T * XS).astype(f8)
    wmod = w * (1.0 / (1.0 + np.exp(-delay)))                   # [D,U,NB]
    h = np.maximum(na[None, :] @ cw1 + cb1, 0.0)
    conn = 1.0 / (1.0 + np.exp(-(h @ cw2 + cb2)))               # [1,U]
    cmw = conn[0] * mask                                        # [U]
    wmod = wmod * cmw[None, :, None]   # fold conn*mask into the weights
    ew = np.exp(actw - actw.max())
    wts = (ew / ew.sum()).astype(np.float32)
    coefs_row = (wts @ _CMAT.astype(np.float64))                     # [5]
    # a arrives C1-scaled (the fp8 descale is folded into the poly):
    # P(a/C1) = sum_i (c_i / C1^i) * a^i
    coefs_row = (coefs_row / (C1 ** np.arange(5))).astype(np.float32)
    coefs_t = np.ascontiguousarray(np.tile(coefs_row, (128, 1)))
    csum_t = np.ascontiguousarray(
        np.tile(mem.sum(axis=0, dtype=np.float64).astype(np.float32)
                * (C4 / RS8), (128, 1)))

    in_maps = []
    for k in range(KC):
        us, ue = k * US, (k + 1) * US
        ms, me = k * MS, (k + 1) * MS
        bias_row = np.concatenate(
            [(b[us:ue] * cmw[us:ue, None]).T.reshape(-1),
             gate_b]).astype(np.float32) * C1
        mem8 = np.concatenate(
            [mem[ms:me] * MSC, np.full((MS, 1), MSC, np.float32),
             np.zeros((MS, GP - 1), np.float32)],
            axis=1).astype(f8)
        in_maps.append({
            "xt": xt8,
            "wm": np.ascontiguousarray(np.concatenate(
                [wmod[:, us:ue, :].transpose(0, 2, 1).reshape(D, UBF) * WS,
                 gate_W * WS, np.zeros((D, GP - NB), np.float32)],
                axis=1)).astype(f8),
            "bias": np.ascontiguousarray(bias_row),
            "coefs": coefs_t,
            "rw": np.ascontiguousarray(read_W[:, ms:me] * RW_SCALE).astype(f8),
            "rb": np.ascontiguousarray(read_b[ms:me]),
            "mem": np.ascontiguousarray(mem8),
            "csum": csum_t,
        })
    return in_maps


def kernel(**inputs) -> np.ndarray:
    with_bias = bool(np.any(np.asarray(inputs["b"]))
                     or np.any(np.asarray(inputs["gate_b"])))
    key = ("nc", with_bias)
    if key not in _cache:
        _cache[key] = _build(with_bias)
        _cache["nc"] = _cache[key]
    nc = _cache[key]
    in_maps = _make_in_maps(inputs)
    res = run_bass_kernel_spmd(nc, in_maps, core_ids=list(range(KC)))
    out = np.empty((N, MD), np.float32)
    for k in range(KC):
        yk = res.results[k]["y"]
        for ch in range(NCH):
            base = 0
            for spp in RS_GROUPS[ch]:
                rows = spp * 128 // KC
                g0 = ch * 512 + base * 128 + k * rows
                l0 = ch * 64 + base * 128 // KC
                out[g0:g0 + rows] = yk[l0:l0 + rows]
                base += spp
    return out
